# revision 4
# baseline (speedup 1.0000x reference)
"""AGRNN message-passing kernel for 8 Trainium2 NeuronCores.

Math (reference):
    nf = relu([n_f_original | new_n_f] @ W_node + b_node)          [N, 1024]
    nl = relu([word2vec | new_n_f_lang] @ W_lang + b_lang)         [N, 300]
    ef = [nf[dst], nl[dst], s_f, nl[src], nf[src]]                 [E, 2664]
    pred = relu(ef @ W_e1 + b_e1) @ W_e2 + b_e2                    [E, 117]

Since the first edge layer is linear, it is pushed through the gather:
    Rd = nf @ W1[0:1024] + nl @ W1[1024:1324]      (node-level, [N, 1024])
    Rs = nl @ W1[1340:1640] + nf @ W1[1640:2664]
    h  = relu(Rd[dst] + Rs[src] + s_f @ W1[1324:1340] + b_e1)
    pred = h @ W_e2 + b_e2
This replaces the [E,2664]@[2664,1024] matmul (546 GFLOP) with node-level
projections (108 GFLOP) plus per-edge gather-adds.

Distribution: nodes sharded 8 ways for the MLPs/projections; the bf16
Rd/Rs tables are all-gathered (split into hid-halves so the collectives
pipeline with compute); edges sharded 8 ways for gather + readout.
"""

import sys

sys.path.insert(0, "/opt/trn_rl_repo")

import numpy as np
import ml_dtypes

import concourse.bacc as bacc
import concourse.mybir as mybir
import concourse.tile as tile
from concourse.bass_utils import run_bass_kernel_spmd

BF16 = ml_dtypes.bfloat16
P = 128


class Cfg:
    def __init__(self, n_nodes, n_edges, d_feat, d_lang, d_sp, d_hid, n_cls,
                 npc, ep, ech=640, hc=2, n_cores=8):
        self.n_cores = n_cores
        self.n_nodes = n_nodes
        self.n_edges = n_edges
        self.d_feat = d_feat
        self.d_lang = d_lang
        self.d_sp = d_sp
        self.d_hid = d_hid
        self.n_cls = n_cls
        self.nodes_pc = n_nodes // n_cores     # real nodes per core
        self.npc = npc                          # padded nodes per core
        self.ep = ep                            # padded edges per core
        self.ech = ech                          # edge compute/gather chunk
        self.hc = hc                            # hid chunks for AG split
        assert npc % 512 == 0 and ep % ech == 0 and ech % P == 0
        self.kf = 2 * d_feat // P               # k-tiles for visual MLP
        assert 2 * d_feat % P == 0
        self.xlk = -(-2 * d_lang // P) * P      # padded lang input dim
        self.kl = self.xlk // P
        self.lang_pad = -(-d_lang // P) * P     # padded lang hidden dim
        self.ml = self.lang_pad // P
        self.mh = d_hid // P
        assert d_hid % (P * hc) == 0
        self.hw = d_hid // hc                   # hid cols per AG chunk
        self.mhc = self.mh // hc
        self.nt = npc // P
        self.nch = npc // 512
        self.nech = ep // ech
        self.esub = ech // P
        self.spk = 32                           # sp matmul K (16 sp + 1 ones + pad)
        assert d_sp + 1 <= self.spk
        # gather rows must be 256B-aligned
        assert (self.hw * 2) % 256 == 0
        assert n_cores * npc < 2 ** 15          # int16 gather indices


FULL_CFG = Cfg(n_nodes=20000, n_edges=100000, d_feat=1024, d_lang=300,
               d_sp=16, d_hid=1024, n_cls=117, npc=2560, ep=12800)


def _n512(total):
    """split a free-dim extent into <=512 matmul chunks"""
    out, o = [], 0
    while o < total:
        w = min(512, total - o)
        out.append((o, w))
        o += w
    return out


def build(cfg: Cfg, phases="123gd"):
    c = cfg
    BF = mybir.dt.bfloat16
    F32 = mybir.dt.float32
    I16 = mybir.dt.int16
    RELU = mybir.ActivationFunctionType.Relu
    ADD = mybir.AluOpType.add

    nc = bacc.Bacc("TRN2", target_bir_lowering=False, debug=False)

    xf = nc.declare_dram_parameter("xf", [P, c.kf, c.npc], BF, isOutput=False)
    xl = nc.declare_dram_parameter("xl", [P, c.kl, c.npc], BF, isOutput=False)
    wn = nc.declare_dram_parameter("wn", [P, c.kf, c.d_hid], BF, isOutput=False)
    bn = nc.declare_dram_parameter("bn", [P, c.mh], F32, isOutput=False)
    wl = nc.declare_dram_parameter("wl", [P, c.kl, c.lang_pad], BF, isOutput=False)
    bl = nc.declare_dram_parameter("bl", [P, c.ml], F32, isOutput=False)
    w1df = nc.declare_dram_parameter("w1df", [P, c.mh, c.d_hid], BF, isOutput=False)
    w1dl = nc.declare_dram_parameter("w1dl", [P, c.ml, c.d_hid], BF, isOutput=False)
    w1sl = nc.declare_dram_parameter("w1sl", [P, c.ml, c.d_hid], BF, isOutput=False)
    w1sf = nc.declare_dram_parameter("w1sf", [P, c.mh, c.d_hid], BF, isOutput=False)
    w1sp = nc.declare_dram_parameter("w1sp", [c.spk, c.d_hid], BF, isOutput=False)
    w2 = nc.declare_dram_parameter("w2", [P, c.mh, c.n_cls], BF, isOutput=False)
    be2 = nc.declare_dram_parameter("be2", [P, c.n_cls], F32, isOutput=False)
    sf = nc.declare_dram_parameter("sf", [c.spk, c.ep], BF, isOutput=False)
    didx = nc.declare_dram_parameter("didx", [P, c.ep // 16], I16, isOutput=False)
    sidx = nc.declare_dram_parameter("sidx", [P, c.ep // 16], I16, isOutput=False)
    out_ext = nc.declare_dram_parameter("out", [c.ep, c.n_cls], F32, isOutput=True)

    rsh, rfull = {}, {}
    for t in "ds":
        for h in range(c.hc):
            rsh[t, h] = nc.dram_tensor(f"rsh_{t}{h}", [c.npc, c.hw], BF)
            rfull[t, h] = nc.dram_tensor(
                f"rfull_{t}{h}", [c.npc * c.n_cores, c.hw], BF, addr_space="Shared"
            )
    RG = [list(range(c.n_cores))]

    with tile.TileContext(nc) as tc:
        with tc.tile_pool(name="pn", bufs=1) as pn:
            nf_t = pn.tile([P, c.mh, c.npc], BF)   # relu(W_node^T @ Xf + b)
            nl_t = pn.tile([P, c.ml, c.npc], BF)
            with tc.tile_pool(name="psA", bufs=4, space="PSUM") as psA:
                # ---- phase A: node MLPs (outputs transposed: hid on partitions)
                with tc.tile_pool(name="pa", bufs=1) as pa:
                  if "1" in phases:
                    xf_sb = pa.tile([P, c.kf, c.npc], BF)
                    nc.sync.dma_start(xf_sb[:], xf[:])
                    wn_sb = pa.tile([P, c.kf, c.d_hid], BF)
                    nc.sync.dma_start(wn_sb[:], wn[:])
                    bn_sb = pa.tile([P, c.mh], F32)
                    nc.sync.dma_start(bn_sb[:], bn[:])
                    for m in range(c.mh):
                        for n in range(c.nch):
                            ps = psA.tile([P, 512], F32)
                            for k in range(c.kf):
                                nc.tensor.matmul(
                                    ps[:],
                                    wn_sb[:, k, m * P:(m + 1) * P],
                                    xf_sb[:, k, n * 512:(n + 1) * 512],
                                    start=(k == 0), stop=(k == c.kf - 1),
                                )
                            nc.scalar.activation(
                                nf_t[:, m, n * 512:(n + 1) * 512], ps[:],
                                RELU, bias=bn_sb[:, m:m + 1],
                            )
                with tc.tile_pool(name="pa2", bufs=1) as pa2:
                  if "2" in phases:
                    xl_sb = pa2.tile([P, c.kl, c.npc], BF)
                    nc.sync.dma_start(xl_sb[:], xl[:])
                    wl_sb = pa2.tile([P, c.kl, c.lang_pad], BF)
                    nc.sync.dma_start(wl_sb[:], wl[:])
                    bl_sb = pa2.tile([P, c.ml], F32)
                    nc.sync.dma_start(bl_sb[:], bl[:])
                    for m in range(c.ml):
                        for n in range(c.nch):
                            ps = psA.tile([P, 512], F32)
                            for k in range(c.kl):
                                nc.tensor.matmul(
                                    ps[:],
                                    wl_sb[:, k, m * P:(m + 1) * P],
                                    xl_sb[:, k, n * 512:(n + 1) * 512],
                                    start=(k == 0), stop=(k == c.kl - 1),
                                )
                            nc.scalar.activation(
                                nl_t[:, m, n * 512:(n + 1) * 512], ps[:],
                                RELU, bias=bl_sb[:, m:m + 1],
                            )
                # ---- phase B: Rd/Rs projections ([nodes, hid] layout) + AllGather
                with tc.tile_pool(name="pb", bufs=1) as pb, \
                     tc.tile_pool(name="pbs", bufs=4) as pbs:
                  if "3" in phases:
                    w1sb = {}
                    for nm, prm, kt in (("df", w1df, c.mh), ("dl", w1dl, c.ml),
                                        ("sl", w1sl, c.ml), ("sf", w1sf, c.mh)):
                        t_ = pb.tile([P, kt, c.d_hid], BF, tag=f"w1{nm}")
                        nc.sync.dma_start(t_[:], prm[:])
                        w1sb[nm] = t_
                    for h in range(c.hc):
                        for t in "ds":
                            wf = w1sb["df" if t == "d" else "sf"]
                            wlng = w1sb["dl" if t == "d" else "sl"]
                            for m in range(c.nt):
                                ps = psA.tile([P, c.hw], F32, tag="psB")
                                for k in range(c.mh):
                                    nc.tensor.matmul(
                                        ps[:],
                                        nf_t[:, k, m * P:(m + 1) * P],
                                        wf[:, k, h * c.hw:(h + 1) * c.hw],
                                        start=(k == 0), stop=False,
                                    )
                                for k in range(c.ml):
                                    nc.tensor.matmul(
                                        ps[:],
                                        nl_t[:, k, m * P:(m + 1) * P],
                                        wlng[:, k, h * c.hw:(h + 1) * c.hw],
                                        start=False, stop=(k == c.ml - 1),
                                    )
                                st = pbs.tile([P, c.hw], BF, tag="st")
                                nc.vector.tensor_copy(st[:], ps[:])
                                nc.sync.dma_start(rsh[t, h][m * P:(m + 1) * P, :], st[:])
                            if "g" in phases:
                                nc.gpsimd.collective_compute(
                                    "AllGather", mybir.AluOpType.bypass,
                                    replica_groups=RG,
                                    ins=[rsh[t, h][:]],
                                    outs=[rfull[t, h][:].opt()],
                                )
        # ---- phase D: edge gather + readout
        with tc.tile_pool(name="pd", bufs=1) as pd, \
             tc.tile_pool(name="pdg", bufs=2) as pdg, \
             tc.tile_pool(name="pdo", bufs=4) as pdo, \
             tc.tile_pool(name="psSP", bufs=2, space="PSUM") as psSP, \
             tc.tile_pool(name="psP", bufs=4, space="PSUM") as psP:
          if "d" in phases:
            sf_sb = pd.tile([c.spk, c.ep], BF)
            nc.sync.dma_start(sf_sb[:], sf[:])
            w1sp_sb = pd.tile([c.spk, c.d_hid], BF)
            nc.sync.dma_start(w1sp_sb[:], w1sp[:])
            w2_sb = pd.tile([P, c.mh, c.n_cls], BF)
            nc.sync.dma_start(w2_sb[:], w2[:])
            be2_sb = pd.tile([P, c.n_cls], F32)
            nc.sync.dma_start(be2_sb[:], be2[:])
            didx_sb = pd.tile([P, c.ep // 16], I16)
            nc.sync.dma_start(didx_sb[:], didx[:])
            sidx_sb = pd.tile([P, c.ep // 16], I16)
            nc.sync.dma_start(sidx_sb[:], sidx[:])

            icol = c.ech // 16
            for ec in range(c.nech):
                htiles = []
                for h in range(c.hc):
                    rdg = pdg.tile([P, c.mhc, c.ech], BF, tag=f"rdg{h}")
                    nc.gpsimd.dma_gather(
                        rdg[:], rfull["d", h][:],
                        didx_sb[:, ec * icol:(ec + 1) * icol],
                        c.ech, c.ech, c.hw, transpose=True,
                    )
                    rsg = pdg.tile([P, c.mhc, c.ech], BF, tag=f"rsg{h}")
                    nc.gpsimd.dma_gather(
                        rsg[:], rfull["s", h][:],
                        sidx_sb[:, ec * icol:(ec + 1) * icol],
                        c.ech, c.ech, c.hw, transpose=True,
                    )
                    hsb = pdg.tile([P, c.mhc, c.ech], BF, tag=f"h{h}")
                    for ml_ in range(c.mhc):
                        m = h * c.mhc + ml_
                        ps = psSP.tile([P, c.ech], F32)
                        for (o, w) in _n512(c.ech):
                            nc.tensor.matmul(
                                ps[:, o:o + w],
                                w1sp_sb[:, m * P:(m + 1) * P],
                                sf_sb[:, ec * c.ech + o:ec * c.ech + o + w],
                                start=True, stop=True,
                            )
                        nc.vector.tensor_tensor(ps[:], ps[:], rdg[:, ml_, :], ADD)
                        nc.vector.tensor_tensor(ps[:], ps[:], rsg[:, ml_, :], ADD)
                        nc.scalar.activation(hsb[:, ml_, :], ps[:], RELU)
                    htiles.append(hsb)
                for es in range(c.esub):
                    pp = psP.tile([P, c.n_cls], F32)
                    for m in range(c.mh):
                        h, ml_ = divmod(m, c.mhc)
                        nc.tensor.matmul(
                            pp[:],
                            htiles[h][:, ml_, es * P:(es + 1) * P],
                            w2_sb[:, m, :],
                            start=(m == 0), stop=(m == c.mh - 1),
                        )
                    ob = pdo.tile([P, c.n_cls], F32, tag="ob")
                    nc.vector.tensor_tensor(ob[:], pp[:], be2_sb[:], ADD)
                    nc.sync.dma_start(
                        out_ext[ec * c.ech + es * P:ec * c.ech + (es + 1) * P, :],
                        ob[:],
                    )

    nc.finalize()
    return nc


def _swizzle_k(w, kpad=None):
    """[K, N] -> [128, K/128, N] (k-tiles on dim 1), zero-padding K to kpad"""
    k, n = w.shape
    if kpad is None:
        kpad = k
    if kpad != k:
        wp = np.zeros((kpad, n), w.dtype)
        wp[:k] = w
        w = wp
    return np.ascontiguousarray(w.reshape(kpad // P, P, n).transpose(1, 0, 2))


def _bias_r(b, pad):
    bp = np.zeros(pad, np.float32)
    bp[:b.shape[0]] = b
    return np.ascontiguousarray(bp.reshape(pad // P, P).T)


def _idx16(ix, c: Cfg):
    """remap node ids to padded-shard table rows; [ep] -> [128, ep/16] int16"""
    g = (ix // c.nodes_pc) * c.npc + ix % c.nodes_pc
    arr = np.ascontiguousarray(g.astype(np.int16).reshape(c.ep // 16, 16).T)
    return np.ascontiguousarray(np.tile(arr, (8, 1)))


def prep_inputs(c: Cfg, inputs):
    n_f_o = np.asarray(inputs["n_f_original"], np.float32)
    new_nf = np.asarray(inputs["new_n_f"], np.float32)
    w2v = np.asarray(inputs["word2vec_original"], np.float32)
    lang = np.asarray(inputs["new_n_f_lang"], np.float32)
    s_f = np.asarray(inputs["s_f"], np.float32)
    W_node = np.asarray(inputs["W_node"], np.float32)
    b_node = np.asarray(inputs["b_node"], np.float32)
    W_lang = np.asarray(inputs["W_lang"], np.float32)
    b_lang = np.asarray(inputs["b_lang"], np.float32)
    W_e1 = np.asarray(inputs["W_e1"], np.float32)
    b_e1 = np.asarray(inputs["b_e1"], np.float32)
    W_e2 = np.asarray(inputs["W_e2"], np.float32)
    b_e2 = np.asarray(inputs["b_e2"], np.float32)
    src = np.asarray(inputs["src_idx"]).astype(np.int64)
    dst = np.asarray(inputs["dst_idx"]).astype(np.int64)

    df, dl = c.d_feat, c.d_lang
    # W_e1 row blocks: [dst_f | dst_l | sp | src_l | src_f]
    W1df = W_e1[0:df]
    W1dl = W_e1[df:df + dl]
    W1sp = W_e1[df + dl:df + dl + c.d_sp]
    W1sl = W_e1[df + dl + c.d_sp:df + 2 * dl + c.d_sp]
    W1sf = W_e1[df + 2 * dl + c.d_sp:]

    shared = {
        "wn": _swizzle_k(W_node.astype(BF16)),
        "bn": _bias_r(b_node, c.d_hid),
        "wl": _swizzle_k(W_lang.astype(BF16), c.xlk)[:, :, :c.d_lang],
        "bl": _bias_r(b_lang, c.lang_pad),
        "w1df": _swizzle_k(W1df.astype(BF16)),
        "w1dl": _swizzle_k(W1dl.astype(BF16), c.lang_pad),
        "w1sl": _swizzle_k(W1sl.astype(BF16), c.lang_pad),
        "w1sf": _swizzle_k(W1sf.astype(BF16)),
        "w2": _swizzle_k(W_e2.astype(BF16)),
        "be2": np.ascontiguousarray(np.broadcast_to(b_e2, (P, c.n_cls))).astype(np.float32),
    }
    # wl above: padded K to xlk then need lang_pad cols; pad columns:
    wlp = np.zeros((P, c.kl, c.lang_pad), BF16)
    wlp[:, :, :c.d_lang] = shared["wl"]
    shared["wl"] = wlp
    # sp weights with ones-row bias fold
    w1sp_pad = np.zeros((c.spk, c.d_hid), np.float32)
    w1sp_pad[:c.d_sp] = W1sp
    w1sp_pad[c.d_sp] = b_e1
    shared["w1sp"] = w1sp_pad.astype(BF16)

    # padded global edge arrays
    etot = c.ep * c.n_cores
    dstp = np.zeros(etot, np.int64)
    dstp[:c.n_edges] = dst
    srcp = np.zeros(etot, np.int64)
    srcp[:c.n_edges] = src
    sfp = np.zeros((etot, c.d_sp), np.float32)
    sfp[:c.n_edges] = s_f

    in_maps = []
    for i in range(c.n_cores):
        ns = slice(i * c.nodes_pc, (i + 1) * c.nodes_pc)
        es = slice(i * c.ep, (i + 1) * c.ep)
        xf_b = np.concatenate([n_f_o[ns], new_nf[ns]], axis=1).astype(BF16)  # [nodes, 2df]
        xf_t = np.zeros((2 * df, c.npc), BF16)
        xf_t[:, :c.nodes_pc] = xf_b.T
        xl_b = np.concatenate([w2v[ns], lang[ns]], axis=1).astype(BF16)
        xl_t = np.zeros((c.xlk, c.npc), BF16)
        xl_t[:2 * dl, :c.nodes_pc] = xl_b.T
        sft = np.zeros((c.spk, c.ep), np.float32)
        sft[:c.d_sp] = sfp[es].T
        sft[c.d_sp] = 1.0
        m = {
            "xf": np.ascontiguousarray(xf_t.reshape(c.kf, P, c.npc).transpose(1, 0, 2)),
            "xl": np.ascontiguousarray(xl_t.reshape(c.kl, P, c.npc).transpose(1, 0, 2)),
            "sf": sft.astype(BF16),
            "didx": _idx16(dstp[es], c),
            "sidx": _idx16(srcp[es], c),
        }
        m.update(shared)
        in_maps.append(m)
    return in_maps


_BUILT = {}


def _get_nc(c: Cfg):
    key = id(c)
    if key not in _BUILT:
        _BUILT[key] = build(c)
    return _BUILT[key]


def run(c: Cfg, inputs, **kw):
    nc = _get_nc(c)
    in_maps = prep_inputs(c, inputs)
    res = run_bass_kernel_spmd(nc, in_maps, list(range(c.n_cores)), **kw)
    out = np.concatenate([r["out"] for r in res.results], axis=0)[:c.n_edges]
    return np.ascontiguousarray(out.astype(np.float32)), res


def kernel(**inputs):
    out, _ = run(FULL_CFG, inputs)
    return out


# revision 5
# speedup vs baseline: 611.1161x; 611.1161x over previous
"""AGRNN message-passing kernel for 8 Trainium2 NeuronCores.

Math (reference):
    nf = relu([n_f_original | new_n_f] @ W_node + b_node)          [N, 1024]
    nl = relu([word2vec | new_n_f_lang] @ W_lang + b_lang)         [N, 300]
    ef = [nf[dst], nl[dst], s_f, nl[src], nf[src]]                 [E, 2664]
    pred = relu(ef @ W_e1 + b_e1) @ W_e2 + b_e2                    [E, 117]

Since the first edge layer is linear, it is pushed through the gather:
    Rd = nf @ W1[0:1024] + nl @ W1[1024:1324]      (node-level, [N, 1024])
    Rs = nl @ W1[1340:1640] + nf @ W1[1640:2664]
    h  = relu(Rd[dst] + Rs[src] + s_f @ W1[1324:1340] + b_e1)
    pred = h @ W_e2 + b_e2
This replaces the [E,2664]@[2664,1024] matmul (546 GFLOP) with node-level
projections (108 GFLOP) plus per-edge gather-adds.

Distribution: nodes sharded 8 ways for the MLPs/projections; the bf16
Rd/Rs tables are all-gathered (split into hid-halves so the collectives
pipeline with compute); edges sharded 8 ways for gather + readout.
"""

import sys

sys.path.insert(0, "/opt/trn_rl_repo")

import numpy as np
import ml_dtypes

import concourse.bacc as bacc
import concourse.mybir as mybir
import concourse.tile as tile
from concourse.bass_utils import run_bass_kernel_spmd

BF16 = ml_dtypes.bfloat16
P = 128


class Cfg:
    def __init__(self, n_nodes, n_edges, d_feat, d_lang, d_sp, d_hid, n_cls,
                 npc, ep, ech=640, hc=2, n_cores=8):
        self.n_cores = n_cores
        self.n_nodes = n_nodes
        self.n_edges = n_edges
        self.d_feat = d_feat
        self.d_lang = d_lang
        self.d_sp = d_sp
        self.d_hid = d_hid
        self.n_cls = n_cls
        self.nodes_pc = n_nodes // n_cores     # real nodes per core
        self.npc = npc                          # padded nodes per core
        self.ep = ep                            # padded edges per core
        self.ech = ech                          # edge compute/gather chunk
        self.hc = hc                            # hid chunks for AG split
        assert npc % 512 == 0 and ep % ech == 0 and ech % P == 0
        self.kf = 2 * d_feat // P               # k-tiles for visual MLP
        assert 2 * d_feat % P == 0
        self.xlk = -(-2 * d_lang // P) * P      # padded lang input dim
        self.kl = self.xlk // P
        self.lang_pad = -(-d_lang // P) * P     # padded lang hidden dim
        self.ml = self.lang_pad // P
        self.mh = d_hid // P
        assert d_hid % (P * hc) == 0
        self.hw = d_hid // hc                   # hid cols per AG chunk
        self.mhc = self.mh // hc
        self.nt = npc // P
        self.nch = npc // 512
        self.nech = ep // ech
        self.esub = ech // P
        self.spk = 32                           # sp matmul K (16 sp + 1 ones + pad)
        assert d_sp + 1 <= self.spk
        # gather rows must be 256B-aligned
        assert (self.hw * 2) % 256 == 0
        assert n_cores * npc < 2 ** 15          # int16 gather indices


FULL_CFG = Cfg(n_nodes=20000, n_edges=100000, d_feat=1024, d_lang=300,
               d_sp=16, d_hid=1024, n_cls=117, npc=2560, ep=12800)


def _n512(total):
    """split a free-dim extent into <=512 matmul chunks"""
    out, o = [], 0
    while o < total:
        w = min(512, total - o)
        out.append((o, w))
        o += w
    return out


def build(cfg: Cfg, phases="123gd", repeat=1):
    c = cfg
    BF = mybir.dt.bfloat16
    F32 = mybir.dt.float32
    I16 = mybir.dt.int16
    RELU = mybir.ActivationFunctionType.Relu
    ADD = mybir.AluOpType.add

    nc = bacc.Bacc("TRN2", target_bir_lowering=False, debug=False)

    xf = nc.declare_dram_parameter("xf", [P, c.kf, c.npc], BF, isOutput=False)
    xl = nc.declare_dram_parameter("xl", [P, c.kl, c.npc], BF, isOutput=False)
    wn = nc.declare_dram_parameter("wn", [P, c.kf, c.d_hid], BF, isOutput=False)
    bn = nc.declare_dram_parameter("bn", [P, c.mh], F32, isOutput=False)
    wl = nc.declare_dram_parameter("wl", [P, c.kl, c.lang_pad], BF, isOutput=False)
    bl = nc.declare_dram_parameter("bl", [P, c.ml], F32, isOutput=False)
    w1df = nc.declare_dram_parameter("w1df", [P, c.mh, c.d_hid], BF, isOutput=False)
    w1dl = nc.declare_dram_parameter("w1dl", [P, c.ml, c.d_hid], BF, isOutput=False)
    w1sl = nc.declare_dram_parameter("w1sl", [P, c.ml, c.d_hid], BF, isOutput=False)
    w1sf = nc.declare_dram_parameter("w1sf", [P, c.mh, c.d_hid], BF, isOutput=False)
    w1sp = nc.declare_dram_parameter("w1sp", [c.spk, c.d_hid], BF, isOutput=False)
    w2 = nc.declare_dram_parameter("w2", [P, c.mh, c.n_cls], BF, isOutput=False)
    be2 = nc.declare_dram_parameter("be2", [P, c.n_cls], F32, isOutput=False)
    sf = nc.declare_dram_parameter("sf", [c.spk, c.ep], BF, isOutput=False)
    didx = nc.declare_dram_parameter("didx", [P, c.ep // 16], I16, isOutput=False)
    sidx = nc.declare_dram_parameter("sidx", [P, c.ep // 16], I16, isOutput=False)
    out_ext = nc.declare_dram_parameter("out", [c.ep, c.n_cls], F32, isOutput=True)

    rsh, rfull = {}, {}
    for t in "ds":
        for h in range(c.hc):
            rsh[t, h] = nc.dram_tensor(f"rsh_{t}{h}", [c.npc, c.hw], BF)
            rfull[t, h] = nc.dram_tensor(
                f"rfull_{t}{h}", [c.npc * c.n_cores, c.hw], BF, addr_space="Shared"
            )
    RG = [list(range(c.n_cores))]

    with tile.TileContext(nc) as tc:
      for _rep in range(repeat):
        with tc.tile_pool(name="pn", bufs=1) as pn:
            nf_t = pn.tile([P, c.mh, c.npc], BF)   # relu(W_node^T @ Xf + b)
            nl_t = pn.tile([P, c.ml, c.npc], BF)
            with tc.tile_pool(name="psA", bufs=4, space="PSUM") as psA:
                # ---- phase A: node MLPs (outputs transposed: hid on partitions)
                with tc.tile_pool(name="pa", bufs=1) as pa:
                  if "1" in phases:
                    xf_sb = pa.tile([P, c.kf, c.npc], BF)
                    nc.sync.dma_start(xf_sb[:], xf[:])
                    wn_sb = pa.tile([P, c.kf, c.d_hid], BF)
                    nc.sync.dma_start(wn_sb[:], wn[:])
                    bn_sb = pa.tile([P, c.mh], F32)
                    nc.sync.dma_start(bn_sb[:], bn[:])
                    for m in range(c.mh):
                        for n in range(c.nch):
                            ps = psA.tile([P, 512], F32)
                            for k in range(c.kf):
                                nc.tensor.matmul(
                                    ps[:],
                                    wn_sb[:, k, m * P:(m + 1) * P],
                                    xf_sb[:, k, n * 512:(n + 1) * 512],
                                    start=(k == 0), stop=(k == c.kf - 1),
                                )
                            nc.scalar.activation(
                                nf_t[:, m, n * 512:(n + 1) * 512], ps[:],
                                RELU, bias=bn_sb[:, m:m + 1],
                            )
                with tc.tile_pool(name="pa2", bufs=1) as pa2:
                  if "2" in phases:
                    xl_sb = pa2.tile([P, c.kl, c.npc], BF)
                    nc.sync.dma_start(xl_sb[:], xl[:])
                    wl_sb = pa2.tile([P, c.kl, c.lang_pad], BF)
                    nc.sync.dma_start(wl_sb[:], wl[:])
                    bl_sb = pa2.tile([P, c.ml], F32)
                    nc.sync.dma_start(bl_sb[:], bl[:])
                    for m in range(c.ml):
                        for n in range(c.nch):
                            ps = psA.tile([P, 512], F32)
                            for k in range(c.kl):
                                nc.tensor.matmul(
                                    ps[:],
                                    wl_sb[:, k, m * P:(m + 1) * P],
                                    xl_sb[:, k, n * 512:(n + 1) * 512],
                                    start=(k == 0), stop=(k == c.kl - 1),
                                )
                            nc.scalar.activation(
                                nl_t[:, m, n * 512:(n + 1) * 512], ps[:],
                                RELU, bias=bl_sb[:, m:m + 1],
                            )
                # ---- phase B: Rd/Rs projections ([nodes, hid] layout) + AllGather
                with tc.tile_pool(name="pb", bufs=1) as pb, \
                     tc.tile_pool(name="pbs", bufs=4) as pbs:
                  if "3" in phases:
                    w1sb = {}
                    for nm, prm, kt in (("df", w1df, c.mh), ("dl", w1dl, c.ml),
                                        ("sl", w1sl, c.ml), ("sf", w1sf, c.mh)):
                        t_ = pb.tile([P, kt, c.d_hid], BF, tag=f"w1{nm}")
                        nc.sync.dma_start(t_[:], prm[:])
                        w1sb[nm] = t_
                    for h in range(c.hc):
                        for t in "ds":
                            wf = w1sb["df" if t == "d" else "sf"]
                            wlng = w1sb["dl" if t == "d" else "sl"]
                            for m in range(c.nt):
                                ps = psA.tile([P, c.hw], F32, tag="psB")
                                for k in range(c.mh):
                                    nc.tensor.matmul(
                                        ps[:],
                                        nf_t[:, k, m * P:(m + 1) * P],
                                        wf[:, k, h * c.hw:(h + 1) * c.hw],
                                        start=(k == 0), stop=False,
                                    )
                                for k in range(c.ml):
                                    nc.tensor.matmul(
                                        ps[:],
                                        nl_t[:, k, m * P:(m + 1) * P],
                                        wlng[:, k, h * c.hw:(h + 1) * c.hw],
                                        start=False, stop=(k == c.ml - 1),
                                    )
                                st = pbs.tile([P, c.hw], BF, tag="st")
                                nc.vector.tensor_copy(st[:], ps[:])
                                nc.sync.dma_start(rsh[t, h][m * P:(m + 1) * P, :], st[:])
                            if "g" in phases:
                                nc.gpsimd.collective_compute(
                                    "AllGather", mybir.AluOpType.bypass,
                                    replica_groups=RG,
                                    ins=[rsh[t, h][:]],
                                    outs=[rfull[t, h][:].opt()],
                                )
        # ---- phase D: edge gather + readout
        with tc.tile_pool(name="pd", bufs=1) as pd, \
             tc.tile_pool(name="pdg", bufs=2) as pdg, \
             tc.tile_pool(name="pdo", bufs=4) as pdo, \
             tc.tile_pool(name="psSP", bufs=2, space="PSUM") as psSP, \
             tc.tile_pool(name="psP", bufs=4, space="PSUM") as psP:
          if "d" in phases:
            sf_sb = pd.tile([c.spk, c.ep], BF)
            nc.sync.dma_start(sf_sb[:], sf[:])
            w1sp_sb = pd.tile([c.spk, c.d_hid], BF)
            nc.sync.dma_start(w1sp_sb[:], w1sp[:])
            w2_sb = pd.tile([P, c.mh, c.n_cls], BF)
            nc.sync.dma_start(w2_sb[:], w2[:])
            be2_sb = pd.tile([P, c.n_cls], F32)
            nc.sync.dma_start(be2_sb[:], be2[:])
            didx_sb = pd.tile([P, c.ep // 16], I16)
            nc.sync.dma_start(didx_sb[:], didx[:])
            sidx_sb = pd.tile([P, c.ep // 16], I16)
            nc.sync.dma_start(sidx_sb[:], sidx[:])

            icol = c.ech // 16
            for ec in range(c.nech):
                htiles = []
                for h in range(c.hc):
                    rdg = pdg.tile([P, c.mhc, c.ech], BF, tag=f"rdg{h}")
                    nc.gpsimd.dma_gather(
                        rdg[:], rfull["d", h][:],
                        didx_sb[:, ec * icol:(ec + 1) * icol],
                        c.ech, c.ech, c.hw, transpose=True,
                    )
                    rsg = pdg.tile([P, c.mhc, c.ech], BF, tag=f"rsg{h}")
                    nc.gpsimd.dma_gather(
                        rsg[:], rfull["s", h][:],
                        sidx_sb[:, ec * icol:(ec + 1) * icol],
                        c.ech, c.ech, c.hw, transpose=True,
                    )
                    hsb = pdg.tile([P, c.mhc, c.ech], BF, tag=f"h{h}")
                    for ml_ in range(c.mhc):
                        m = h * c.mhc + ml_
                        ps = psSP.tile([P, c.ech], F32)
                        for (o, w) in _n512(c.ech):
                            nc.tensor.matmul(
                                ps[:, o:o + w],
                                w1sp_sb[:, m * P:(m + 1) * P],
                                sf_sb[:, ec * c.ech + o:ec * c.ech + o + w],
                                start=True, stop=True,
                            )
                        nc.vector.tensor_tensor(ps[:], ps[:], rdg[:, ml_, :], ADD)
                        nc.vector.tensor_tensor(ps[:], ps[:], rsg[:, ml_, :], ADD)
                        nc.scalar.activation(hsb[:, ml_, :], ps[:], RELU)
                    htiles.append(hsb)
                for es in range(c.esub):
                    pp = psP.tile([P, c.n_cls], F32)
                    for m in range(c.mh):
                        h, ml_ = divmod(m, c.mhc)
                        nc.tensor.matmul(
                            pp[:],
                            htiles[h][:, ml_, es * P:(es + 1) * P],
                            w2_sb[:, m, :],
                            start=(m == 0), stop=(m == c.mh - 1),
                        )
                    ob = pdo.tile([P, c.n_cls], F32, tag="ob")
                    nc.vector.tensor_tensor(ob[:], pp[:], be2_sb[:], ADD)
                    nc.sync.dma_start(
                        out_ext[ec * c.ech + es * P:ec * c.ech + (es + 1) * P, :],
                        ob[:],
                    )

    nc.finalize()
    return nc


def _swizzle_k(w, kpad=None):
    """[K, N] -> [128, K/128, N] (k-tiles on dim 1), zero-padding K to kpad"""
    k, n = w.shape
    if kpad is None:
        kpad = k
    if kpad != k:
        wp = np.zeros((kpad, n), w.dtype)
        wp[:k] = w
        w = wp
    return np.ascontiguousarray(w.reshape(kpad // P, P, n).transpose(1, 0, 2))


def _bias_r(b, pad):
    bp = np.zeros(pad, np.float32)
    bp[:b.shape[0]] = b
    return np.ascontiguousarray(bp.reshape(pad // P, P).T)


def _idx16(ix, c: Cfg):
    """remap node ids to padded-shard table rows; [ep] -> [128, ep/16] int16"""
    g = (ix // c.nodes_pc) * c.npc + ix % c.nodes_pc
    arr = np.ascontiguousarray(g.astype(np.int16).reshape(c.ep // 16, 16).T)
    return np.ascontiguousarray(np.tile(arr, (8, 1)))


def prep_inputs(c: Cfg, inputs):
    n_f_o = np.asarray(inputs["n_f_original"], np.float32)
    new_nf = np.asarray(inputs["new_n_f"], np.float32)
    w2v = np.asarray(inputs["word2vec_original"], np.float32)
    lang = np.asarray(inputs["new_n_f_lang"], np.float32)
    s_f = np.asarray(inputs["s_f"], np.float32)
    W_node = np.asarray(inputs["W_node"], np.float32)
    b_node = np.asarray(inputs["b_node"], np.float32)
    W_lang = np.asarray(inputs["W_lang"], np.float32)
    b_lang = np.asarray(inputs["b_lang"], np.float32)
    W_e1 = np.asarray(inputs["W_e1"], np.float32)
    b_e1 = np.asarray(inputs["b_e1"], np.float32)
    W_e2 = np.asarray(inputs["W_e2"], np.float32)
    b_e2 = np.asarray(inputs["b_e2"], np.float32)
    src = np.asarray(inputs["src_idx"]).astype(np.int64)
    dst = np.asarray(inputs["dst_idx"]).astype(np.int64)

    df, dl = c.d_feat, c.d_lang
    # W_e1 row blocks: [dst_f | dst_l | sp | src_l | src_f]
    W1df = W_e1[0:df]
    W1dl = W_e1[df:df + dl]
    W1sp = W_e1[df + dl:df + dl + c.d_sp]
    W1sl = W_e1[df + dl + c.d_sp:df + 2 * dl + c.d_sp]
    W1sf = W_e1[df + 2 * dl + c.d_sp:]

    shared = {
        "wn": _swizzle_k(W_node.astype(BF16)),
        "bn": _bias_r(b_node, c.d_hid),
        "wl": _swizzle_k(W_lang.astype(BF16), c.xlk)[:, :, :c.d_lang],
        "bl": _bias_r(b_lang, c.lang_pad),
        "w1df": _swizzle_k(W1df.astype(BF16)),
        "w1dl": _swizzle_k(W1dl.astype(BF16), c.lang_pad),
        "w1sl": _swizzle_k(W1sl.astype(BF16), c.lang_pad),
        "w1sf": _swizzle_k(W1sf.astype(BF16)),
        "w2": _swizzle_k(W_e2.astype(BF16)),
        "be2": np.ascontiguousarray(np.broadcast_to(b_e2, (P, c.n_cls))).astype(np.float32),
    }
    # wl above: padded K to xlk then need lang_pad cols; pad columns:
    wlp = np.zeros((P, c.kl, c.lang_pad), BF16)
    wlp[:, :, :c.d_lang] = shared["wl"]
    shared["wl"] = wlp
    # sp weights with ones-row bias fold
    w1sp_pad = np.zeros((c.spk, c.d_hid), np.float32)
    w1sp_pad[:c.d_sp] = W1sp
    w1sp_pad[c.d_sp] = b_e1
    shared["w1sp"] = w1sp_pad.astype(BF16)

    # padded global edge arrays
    etot = c.ep * c.n_cores
    dstp = np.zeros(etot, np.int64)
    dstp[:c.n_edges] = dst
    srcp = np.zeros(etot, np.int64)
    srcp[:c.n_edges] = src
    sfp = np.zeros((etot, c.d_sp), np.float32)
    sfp[:c.n_edges] = s_f

    in_maps = []
    for i in range(c.n_cores):
        ns = slice(i * c.nodes_pc, (i + 1) * c.nodes_pc)
        es = slice(i * c.ep, (i + 1) * c.ep)
        xf_b = np.concatenate([n_f_o[ns], new_nf[ns]], axis=1).astype(BF16)  # [nodes, 2df]
        xf_t = np.zeros((2 * df, c.npc), BF16)
        xf_t[:, :c.nodes_pc] = xf_b.T
        xl_b = np.concatenate([w2v[ns], lang[ns]], axis=1).astype(BF16)
        xl_t = np.zeros((c.xlk, c.npc), BF16)
        xl_t[:2 * dl, :c.nodes_pc] = xl_b.T
        sft = np.zeros((c.spk, c.ep), np.float32)
        sft[:c.d_sp] = sfp[es].T
        sft[c.d_sp] = 1.0
        m = {
            "xf": np.ascontiguousarray(xf_t.reshape(c.kf, P, c.npc).transpose(1, 0, 2)),
            "xl": np.ascontiguousarray(xl_t.reshape(c.kl, P, c.npc).transpose(1, 0, 2)),
            "sf": sft.astype(BF16),
            "didx": _idx16(dstp[es], c),
            "sidx": _idx16(srcp[es], c),
        }
        m.update(shared)
        in_maps.append(m)
    return in_maps


_BUILT = {}


def _get_nc(c: Cfg):
    key = id(c)
    if key not in _BUILT:
        _BUILT[key] = build(c)
    return _BUILT[key]


def run(c: Cfg, inputs, **kw):
    nc = _get_nc(c)
    in_maps = prep_inputs(c, inputs)
    res = run_bass_kernel_spmd(nc, in_maps, list(range(c.n_cores)), **kw)
    out = np.concatenate([r["out"] for r in res.results], axis=0)[:c.n_edges]
    return np.ascontiguousarray(out.astype(np.float32)), res


def kernel(**inputs):
    out, _ = run(FULL_CFG, inputs)
    return out
